# revision 4
# baseline (speedup 1.0000x reference)
"""Trainium2 Bass kernel for nn_AttentionModule_30021821399395 — v4.

Math (per token, head h; C=64 channels, degree-1 exp poly c0 + c1*s):
  attn[c_q] = N/Z with N = c0*P0 + c1*Q o P1, Z = c0(q+1) + c1*Q o S1,
  (P0, P1, S1) = prefix sums over k<=c_q of (V, K*V, K).

v4 delta-decomposition: split attn = A0 + delta where
  A0[c_q] = mean_{k<=c_q} V[k]  (the dominant ~93% part) is LINEAR in V, so
  y_A0 = x @ Wvo with Wvo = Wv @ blockdiag(M)^T @ Wo precomputed on host
  (M[q,k] = [k<=q]/(q+1)).  The on-chip work is only the small correction
  delta = c1 * Q o (P1 - P0 o S1/(q+1)) / Z  (~7% magnitude), for which
  1/Z ~= 1/(c0(q+1)) (a <=5% perturbation of a 7% term; validated 4.6e-3).

Everything then runs in fp8 E4M3 DoubleRow (0.5 cyc/row vs fp16's 1.0):
  - Q/K/V projections: 1 fp8 stream each, drained by ONE wide fp8 copy
    (scales folded into host weights so Q8=32Qs, K8=32c1K, V8=V share
    one 0.5 copy scale).
  - prefix sums: one fp8 DR matmul each with a [tri01; triN64] two-subtile
    stationary; the unused subtile reads a static zero region via a
    stride-surgery AP.  Per-channel scale vectors sqrt(g)*ca and g/32 on
    the PSUM drains make m = S1d*P0d and d = P1d - m come out at
    delta8 = 32*delta directly (g = 1/(c0(q+1))).
  - out: y = x@wvo_hi + x@wvo_lo + xr8@wvo_hi + delta8@wo8 accumulated in
    one PSUM bank (A0 needs 3 streams: 2-stream fails at 2.7e-2).  Output
    is WVOS-scaled fp16; host divides by WVOS.

Schedule: 16 slots of (t,cch) projection pipeline; slot s also carries one
out-projection piece: slots 1..8 precompute A0 for token-chunk t=1 into
SBUF (a016), slots 9..16 run fused A0+delta groups for t=0, and the tail
runs t=1's delta matmuls + DVE add with a016.

Sharding: data-parallel over the 8192 tokens -> 1024 tokens/core x 8.
numpy oracle rel err 4.63e-3 (budget 2e-2).
"""

import sys

if "/opt/trn_rl_repo" not in sys.path:
    sys.path.insert(0, "/opt/trn_rl_repo")

import numpy as np

B, S, D = 4, 2048, 1024
H, C = 16, 64
HID = H * C
NCORES = 8
TOK = B * S
TPC = TOK // NCORES         # 1024 tokens per core
TCH = 512                   # token chunk
NT = TPC // TCH             # 2
NCH = HID // 128            # 8 hid chunks
ND = D // 128               # 8 contraction chunks

COEF = np.array([1.0013245, 1.0040334], dtype=np.float64)
C0, C1 = float(COEF[0]), float(COEF[1])
WVOS = 4096.0

_CACHE = {}


def _sub2(ap, stride):
    """[128, N] AP -> [128, 2, N] with a custom middle-dim stride, selecting
    the real data subtile and a static-zero subtile for DoubleRow."""
    a = list(ap.ap)
    assert len(a) == 2, a
    new = [list(a[0]), [stride, 2], list(a[1])]
    return type(ap)(ap.tensor, ap.offset, new)


def _build_bass():
    import concourse.mybir as mybir
    import concourse.tile as tile
    from concourse import bacc

    f32 = mybir.dt.float32
    f16 = mybir.dt.float16
    f8 = mybir.dt.float8e4
    DR = mybir.MatmulPerfMode.DoubleRow

    nc = bacc.Bacc("TRN2")

    xt8 = nc.dram_tensor("xt8", [D, TPC], f8, kind="ExternalInput")
    xtr8 = nc.dram_tensor("xtr8", [D, TPC], f8, kind="ExternalInput")
    wq8 = nc.dram_tensor("wq8", [D, HID], f8, kind="ExternalInput")
    wk8 = nc.dram_tensor("wk8", [D, HID], f8, kind="ExternalInput")
    wv8 = nc.dram_tensor("wv8", [D, HID], f8, kind="ExternalInput")
    wvoh = nc.dram_tensor("wvoh", [D, D], f8, kind="ExternalInput")
    wvol = nc.dram_tensor("wvol", [D, D], f8, kind="ExternalInput")
    wo8 = nc.dram_tensor("wo8", [HID, D], f8, kind="ExternalInput")
    out_t = nc.dram_tensor("out_t", [D, TPC], f16, kind="ExternalOutput")

    # two-subtile DR stationary: sub0 = tri01 (k<=q), sub1 = 64/(q+1)*tri01,
    # both 2-head block-diagonal over the 128 partitions.
    tri01 = np.triu(np.ones((C, C), np.float32))          # [k, q]
    qp1 = np.arange(1, C + 1, dtype=np.float32)
    triN = tri01 * (64.0 / qp1)[None, :]

    def blk(m):
        b = np.zeros((128, 128), np.float32)
        b[:C, :C] = m
        b[C:, C:] = m
        return b

    import ml_dtypes
    tri_np = np.stack([blk(tri01), blk(triN)]).astype(ml_dtypes.float8_e4m3)
    tri_d = nc.inline_tensor(tri_np, name="tri")

    g = 1.0 / (C0 * qp1.astype(np.float64))
    ca = np.sqrt(1.0 / 2048.0)
    sc1_np = np.tile((np.sqrt(g) * ca).astype(np.float32), 2).reshape(128, 1)
    sc2_np = np.tile((g / 32.0).astype(np.float32), 2).reshape(128, 1)
    sc1_d = nc.inline_tensor(sc1_np, name="sc1")
    sc2_d = nc.inline_tensor(sc2_np, name="sc2")

    with tile.TileContext(nc) as tc:
        with (
            tc.tile_pool(name="res", bufs=1) as res,
            tc.tile_pool(name="qkv8p", bufs=2) as qkv8p,
            tc.tile_pool(name="kv8p", bufs=2) as kv8p,
            tc.tile_pool(name="trp", bufs=3) as trp,
            tc.tile_pool(name="mdp", bufs=3) as mdp,
            tc.tile_pool(name="osb", bufs=6) as osbp,
            tc.tile_pool(name="psQKV", bufs=1, space="PSUM") as psQKV,   # 3 banks
            tc.tile_pool(name="psTri", bufs=1, space="PSUM") as psTri,   # 3 banks
            tc.tile_pool(name="psY", bufs=2, space="PSUM") as psY,       # 2 banks
        ):
            # ---- resident loads (SP queue), ordered by first use.
            def load3(dst, dram_ap, eng=None):
                (eng or nc.sync).dma_start(
                    dst[:, :, :],
                    dram_ap.rearrange("(dc p) f -> p dc f", p=128),
                )

            def load3h(dst, dram_ap, half, eng=None):
                F = dst.shape[2]
                fsl = slice(half * F // 2, (half + 1) * F // 2)
                (eng or nc.sync).dma_start(
                    dst[:, :, fsl],
                    dram_ap.rearrange("(dc p) f -> p dc f", p=128)[:, :, fsl],
                )

            def load3q(dst, dram_ap, half, dclo):
                F = dst.shape[2]
                fsl = slice(half * F // 2, (half + 1) * F // 2)
                dsl = slice(0, ND // 2) if dclo else slice(ND // 2, ND)
                nc.sync.dma_start(
                    dst[:, dsl, fsl],
                    dram_ap.rearrange("(dc p) f -> p dc f", p=128)[:, dsl, fsl],
                )

            x8_sb = res.tile([128, ND, TPC], f8, tag="x8", name="x8")
            xr8_sb = res.tile([128, ND, TPC], f8, tag="xr8", name="xr8")
            wq_sb = res.tile([128, ND, HID], f8, tag="wq", name="wq")
            wk_sb = res.tile([128, ND, HID], f8, tag="wk", name="wk")
            wv_sb = res.tile([128, ND, HID], f8, tag="wv", name="wv")
            wvoh_sb = res.tile([128, ND, D], f8, tag="wvoh", name="wvoh")
            wvol_sb = res.tile([128, ND, D], f8, tag="wvol", name="wvol")
            wo8_sb = res.tile([128, ND, D], f8, tag="wo8", name="wo8")

            # iteration-0 critical set first (quarter DMAs)
            load3q(x8_sb, xt8[:, :], 0, True)
            load3q(wq_sb, wq8[:, :], 0, True)
            load3q(x8_sb, xt8[:, :], 0, False)
            load3q(wq_sb, wq8[:, :], 0, False)
            load3h(wk_sb, wk8[:, :], 0)
            load3h(wv_sb, wv8[:, :], 0)
            # consts for the trio matmuls (needed slot 1)
            tri_sb = res.tile([128, 2, 128], f8, tag="tri", name="tri_sb")
            nc.sync.dma_start(tri_sb[:, :, :],
                              tri_d[:, :, :].rearrange("s k q -> k s q"))
            sc1_sb = res.tile([128, 1], f32, tag="sc1", name="sc1_sb")
            nc.sync.dma_start(sc1_sb[:], sc1_d[:, :])
            sc2_sb = res.tile([128, 1], f32, tag="sc2", name="sc2_sb")
            nc.sync.dma_start(sc2_sb[:], sc2_d[:, :])
            # rest of the projection weights
            load3h(wq_sb, wq8[:, :], 1)
            load3h(wk_sb, wk8[:, :], 1)
            load3h(wv_sb, wv8[:, :], 1)
            # t=1 A0 needs: x8-t1, wvoh, wvol, xr8-t1
            load3h(x8_sb, xt8[:, :], 1)
            load3(wvoh_sb, wvoh[:, :])
            load3(wvol_sb, wvol[:, :])
            load3h(xr8_sb, xtr8[:, :], 1)
            load3h(xr8_sb, xtr8[:, :], 0)
            load3(wo8_sb, wo8[:, :])

            # delta tiles per t-chunk, a016 for the t=1 A0 precompute
            delta_sb = [
                res.tile([128, NCH, TCH], f8, tag=f"d{t}", name=f"delta{t}")
                for t in range(NT)
            ]
            a016 = res.tile([128, ND, TCH], f16, tag="a016", name="a016")

            # qkv8 / kv8 ping-pong tiles with static zero pads: grab both
            # rotations once, memset the pads, then rotate normally.
            boot_qkv = [qkv8p.tile([128, 2048], f8, tag="qkv8", name=f"qkvb{i}")
                        for i in range(2)]
            boot_kv = [kv8p.tile([128, 1024], f8, tag="kv8", name=f"kvb{i}")
                       for i in range(2)]
            for bt in boot_qkv:
                nc.gpsimd.memset(bt[:, 1536:2048], 0)
            for bt in boot_kv:
                nc.gpsimd.memset(bt[:, 512:1024], 0)

            def stage_proj(t, cch):
                """Q/K/V fp8 DR projections -> one [Q|K|V] 3-bank PSUM tile."""
                tsl = slice(t * TCH, (t + 1) * TCH)
                csl = slice(cch * 128, (cch + 1) * 128)
                qp = psQKV.tile([128, 3 * TCH], f32, tag="qkv", name="qkv_ps")
                for half, wsb in ((0, wq_sb), (1, wk_sb), (2, wv_sb)):
                    for j, dc in enumerate(range(0, ND, 2)):
                        nc.tensor.matmul(
                            qp[:, half * TCH:(half + 1) * TCH],
                            lhsT=wsb[:, dc:dc + 2, csl],
                            rhs=x8_sb[:, dc:dc + 2, tsl],
                            start=(j == 0),
                            stop=(j == ND // 2 - 1),
                            perf_mode=DR,
                        )
                return qp

            def stage_qkv_copy(qp):
                """One wide fp8 drain: [Q8|K8|V8] = 0.5 * psum."""
                qt = qkv8p.tile([128, 2048], f8, tag="qkv8", name="qkv8")
                nc.scalar.mul(qt[:, 0:1536], qp[:], 0.5)
                return qt

            def stage_kv(qt):
                kt = kv8p.tile([128, 1024], f8, tag="kv8", name="kv8")
                nc.gpsimd.tensor_mul(kt[:, 0:512], qt[:, 512:1024], qt[:, 1024:1536])
                return kt

            def stage_trio(qt, kt):
                """[S1|P0|P1] prefix sums: 3 fp8 DR matmuls, each using one
                subtile of the [tri01; triN64] stationary via zero-pads."""
                tp = psTri.tile([128, 3 * TCH], f32, tag="tri", name="tri_ps")
                # S1 = tri01 @ K8: sub0 = K8 (cols 512:1024), sub1 = zeros (1536:)
                nc.tensor.matmul(
                    tp[:, 0:TCH], lhsT=tri_sb[:, :, :],
                    rhs=_sub2(qt[:, 512:1024], 1024),
                    start=True, stop=True, perf_mode=DR,
                )
                # P0 = triN64 @ V8: sub0 = zeros (1536:), sub1 = V8 (1024:1536)
                nc.tensor.matmul(
                    tp[:, TCH:2 * TCH], lhsT=tri_sb[:, :, :],
                    rhs=_sub2(qt[:, 1536:2048], -512),
                    start=True, stop=True, perf_mode=DR,
                )
                # P1 = tri01 @ kv8: sub0 = kv8, sub1 = zeros (512:1024)
                nc.tensor.matmul(
                    tp[:, 2 * TCH:3 * TCH], lhsT=tri_sb[:, :, :],
                    rhs=_sub2(kt[:, 0:512], 512),
                    start=True, stop=True, perf_mode=DR,
                )
                return tp

            def stage_trio_copy(tp):
                """Scaled fp16 drains: [S1d|P0d] = sc1*psum (DVE),
                P1d = sc2*psum (ACT)."""
                t1 = trp.tile([128, 2 * TCH], f16, tag="t1", name="t1")
                nc.vector.tensor_scalar_mul(t1[:], tp[:, 0:2 * TCH], sc1_sb[:, 0:1])
                t2 = trp.tile([128, TCH], f16, tag="t2", name="t2")
                nc.scalar.mul(t2[:], tp[:, 2 * TCH:3 * TCH], sc2_sb[:, 0:1])
                return t1, t2

            def stage_md(t1, t2):
                mm = mdp.tile([128, TCH], f16, tag="mm", name="mm")
                nc.vector.tensor_mul(mm[:], t1[:, 0:TCH], t1[:, TCH:2 * TCH])
                dd = mdp.tile([128, TCH], f16, tag="dd", name="dd")
                nc.vector.tensor_sub(dd[:], t2[:], mm[:])
                return dd

            def stage_delta(t, cch, qt, dd):
                """delta8 = Q8 * d  (Pool, fp8 out) into the per-t tile."""
                nc.gpsimd.tensor_mul(delta_sb[t][:, cch, :], qt[:, 0:512], dd[:])

            def a0_mms(t, dc, yp, stop):
                """x8@wvo_hi + x8@wvo_lo + xr8@wvo_hi into yp."""
                tsl = slice(t * TCH, (t + 1) * TCH)
                dcsl = slice(dc * 128, (dc + 1) * 128)
                streams = ((wvoh_sb, x8_sb), (wvol_sb, x8_sb), (wvoh_sb, xr8_sb))
                for si, (wsb, xs) in enumerate(streams):
                    for j, dcc in enumerate(range(0, ND, 2)):
                        nc.tensor.matmul(
                            yp[:],
                            lhsT=wsb[:, dcc:dcc + 2, dcsl],
                            rhs=xs[:, dcc:dcc + 2, tsl],
                            start=(si == 0 and j == 0),
                            stop=(stop and si == 2 and j == ND // 2 - 1),
                            perf_mode=DR,
                        )

            def delta_mms(t, dc, yp, start):
                dcsl = slice(dc * 128, (dc + 1) * 128)
                for j, hc in enumerate(range(0, NCH, 2)):
                    nc.tensor.matmul(
                        yp[:],
                        lhsT=wo8_sb[:, hc:hc + 2, dcsl],
                        rhs=delta_sb[t][:, hc:hc + 2, :],
                        start=(start and j == 0),
                        stop=(j == NCH // 2 - 1),
                        perf_mode=DR,
                    )

            def piece_a0_sbuf(dc):
                """Precompute A0 for t=1 into a016[dc] (tail uses DVE add)."""
                yp = psY.tile([128, TCH], f32, tag="y", name="y_a0")
                a0_mms(1, dc, yp, stop=True)
                nc.scalar.copy(a016[:, dc, :], yp[:])

            def piece_y_fused(dc, alt):
                """Full fused out group for t=0: A0 + delta -> copy -> DMA."""
                yp = psY.tile([128, TCH], f32, tag="y", name="y_f")
                a0_mms(0, dc, yp, stop=False)
                delta_mms(0, dc, yp, start=False)
                ot = osbp.tile([128, TCH], f16, tag="ot", name="ot")
                if alt:
                    nc.scalar.copy(ot[:], yp[:])
                else:
                    nc.vector.tensor_copy(ot[:], yp[:])
                deng = nc.sync if alt else nc.scalar
                deng.dma_start(out_t[dc * 128:(dc + 1) * 128, 0:TCH], ot[:])

            def piece_y_tail(dc):
                yp = psY.tile([128, TCH], f32, tag="y", name="y_t")
                delta_mms(1, dc, yp, start=True)
                ot = osbp.tile([128, TCH], f16, tag="ot", name="ot")
                nc.vector.tensor_add(ot[:], yp[:], a016[:, dc, :])
                deng = nc.sync if dc % 2 == 0 else nc.scalar
                deng.dma_start(out_t[dc * 128:(dc + 1) * 128, TCH:2 * TCH], ot[:])

            # ---- main pipeline ----
            its = [(t, cch) for t in range(NT) for cch in range(NCH)]
            n = len(its)
            ctx = {}
            for idx in range(n + 1):
                if idx < n:
                    qp = stage_proj(*its[idx])
                    qt = stage_qkv_copy(qp)
                    kt = stage_kv(qt)
                    ctx[idx] = (qt, kt)
                if idx >= 1:
                    pqt, pkt = ctx.pop(idx - 1)
                    tp = stage_trio(pqt, pkt)
                    t1, t2 = stage_trio_copy(tp)
                    dd = stage_md(t1, t2)
                    t_, cch_ = its[idx - 1]
                    stage_delta(t_, cch_, pqt, dd)
                # out-projection piece for this slot
                if 1 <= idx <= 8:
                    piece_a0_sbuf(idx - 1)
                elif 9 <= idx <= 16:
                    piece_y_fused(idx - 9, alt=(idx % 2 == 0))
            for dc in range(ND):
                piece_y_tail(dc)

    nc.finalize()
    return nc


def _get_nc():
    if "nc" not in _CACHE:
        _CACHE["nc"] = _build_bass()
    return _CACHE["nc"]


def _e4m3(x):
    import ml_dtypes
    return np.clip(x, -240.0, 240.0).astype(ml_dtypes.float8_e4m3)


def _make_in_maps(x, Wq, bq, Wk, bk, Wv, bv, Wo, bo):
    for b in (bq, bk, bv, bo):
        assert not np.any(np.asarray(b)), "nonzero biases not supported"
    x_flat = np.ascontiguousarray(x, dtype=np.float32).reshape(TOK, D)
    Wq = np.ascontiguousarray(Wq, dtype=np.float32)
    Wk = np.ascontiguousarray(Wk, dtype=np.float32)
    Wv = np.ascontiguousarray(Wv, dtype=np.float32)
    Wo = np.ascontiguousarray(Wo, dtype=np.float32)

    # Wvo = Wv @ blockdiag(M)^T @ Wo, M[q,k] = [k<=q]/(q+1)
    M = np.tril(np.ones((C, C), np.float64)) / np.arange(1, C + 1)[:, None]
    BD = np.zeros((HID, HID), np.float64)
    for h in range(H):
        BD[h * C:(h + 1) * C, h * C:(h + 1) * C] = M.T
    Wvo = (Wv.astype(np.float64) @ (BD @ Wo.astype(np.float64))).astype(np.float32)

    wq8 = _e4m3(8.0 * Wq)
    wk8 = _e4m3(64.0 * C1 * Wk)
    wv8 = _e4m3(2.0 * Wv)
    wvoh = _e4m3(WVOS * Wvo)
    wvol = _e4m3(WVOS * Wvo - wvoh.astype(np.float32))
    wo8 = _e4m3((WVOS / 32.0) * Wo)

    in_maps = []
    for i in range(NCORES):
        shard = x_flat[i * TPC:(i + 1) * TPC]
        xt = np.ascontiguousarray(shard.T)
        xt8 = _e4m3(xt)
        xtr8 = _e4m3(xt - xt8.astype(np.float32))
        in_maps.append({
            "xt8": xt8, "xtr8": xtr8,
            "wq8": wq8, "wk8": wk8, "wv8": wv8,
            "wvoh": wvoh, "wvol": wvol, "wo8": wo8,
        })
    return in_maps


def _run(in_maps, trace=False, **kw):
    from concourse import bass_utils
    nc = _get_nc()
    res = bass_utils.run_bass_kernel_spmd(
        nc, in_maps, core_ids=list(range(NCORES)), trace=trace, **kw
    )
    return res


def kernel(x, Wq, bq, Wk, bk, Wv, bv, Wo, bo):
    in_maps = _make_in_maps(x, Wq, bq, Wk, bk, Wv, bv, Wo, bo)
    out = np.empty((TOK, D), np.float32)
    for attempt in range(2):
        res = _run(in_maps, trace=False)
        for i in range(NCORES):
            out[i * TPC:(i + 1) * TPC] = (
                res.results[i]["out_t"].T.astype(np.float32) / WVOS
            )
        if np.isfinite(out).all():
            break
    return out.reshape(B, S, D)


# revision 30
# speedup vs baseline: 1.1437x; 1.1437x over previous
"""Trainium2 Bass kernel for nn_AttentionModule_30021821399395 — v4.

Math (per token, head h; C=64 channels, degree-1 exp poly c0 + c1*s):
  attn[c_q] = N/Z with N = c0*P0 + c1*Q o P1, Z = c0(q+1) + c1*Q o S1,
  (P0, P1, S1) = prefix sums over k<=c_q of (V, K*V, K).

v4 delta-decomposition: split attn = A0 + delta where
  A0[c_q] = mean_{k<=c_q} V[k]  (the dominant ~93% part) is LINEAR in V, so
  y_A0 = x @ Wvo with Wvo = Wv @ blockdiag(M)^T @ Wo precomputed on host
  (M[q,k] = [k<=q]/(q+1)).  The on-chip work is only the small correction
  delta = c1 * Q o (P1 - P0 o S1/(q+1)) / Z  (~7% magnitude), for which
  1/Z ~= 1/(c0(q+1)) (a <=5% perturbation of a 7% term; validated 4.6e-3).

Everything then runs in fp8 E4M3 DoubleRow (0.5 cyc/row vs fp16's 1.0):
  - Q/K/V projections: 1 fp8 stream each, drained by ONE wide fp8 copy
    (scales folded into host weights so Q8=32Qs, K8=32c1K, V8=V share
    one 0.5 copy scale).
  - prefix sums: one fp8 DR matmul each with a [tri01; triN64] two-subtile
    stationary; the unused subtile reads a static zero region via a
    stride-surgery AP.  Per-channel scale vectors sqrt(g)*ca and g/32 on
    the PSUM drains make m = S1d*P0d and d = P1d - m come out at
    delta8 = 32*delta directly (g = 1/(c0(q+1))).
  - out: y = x@wvo_hi + x@wvo_lo + xr8@wvo_hi + delta8@wo8 accumulated in
    one PSUM bank (A0 needs 3 streams: 2-stream fails at 2.7e-2).  Output
    is WVOS-scaled fp16; host divides by WVOS.

Schedule: 16 slots of (t,cch) projection pipeline; slot s also carries one
out-projection piece: slots 1..8 precompute A0 for token-chunk t=1 into
SBUF (a016), slots 9..16 run fused A0+delta groups for t=0, and the tail
runs t=1's delta matmuls + DVE add with a016.

Sharding: data-parallel over the 8192 tokens -> 1024 tokens/core x 8.
numpy oracle rel err 4.63e-3 (budget 2e-2).
"""

import sys

if "/opt/trn_rl_repo" not in sys.path:
    sys.path.insert(0, "/opt/trn_rl_repo")

import numpy as np

B, S, D = 4, 2048, 1024
H, C = 16, 64
HID = H * C
NCORES = 8
TOK = B * S
TPC = TOK // NCORES         # 1024 tokens per core
TCH = 512                   # token chunk
NT = TPC // TCH             # 2
NCH = HID // 128            # 8 hid chunks
ND = D // 128               # 8 contraction chunks

COEF = np.array([1.0013245, 1.0040334], dtype=np.float64)
C0, C1 = float(COEF[0]), float(COEF[1])
WVOS = 4096.0

_CACHE = {}


def _sub2(ap, stride):
    """[128, N] AP -> [128, 2, N] with a custom middle-dim stride, selecting
    the real data subtile and a static-zero subtile for DoubleRow."""
    a = list(ap.ap)
    assert len(a) == 2, a
    new = [list(a[0]), [stride, 2], list(a[1])]
    return type(ap)(ap.tensor, ap.offset, new)


def _build_bass():
    import concourse.mybir as mybir
    import concourse.tile as tile
    from concourse import bacc

    f32 = mybir.dt.float32
    f16 = mybir.dt.float16
    f8 = mybir.dt.float8e4
    DR = mybir.MatmulPerfMode.DoubleRow

    nc = bacc.Bacc("TRN2")

    xt8 = nc.dram_tensor("xt8", [D, TPC], f8, kind="ExternalInput")
    xtr8 = nc.dram_tensor("xtr8", [D, TPC], f8, kind="ExternalInput")
    wq8 = nc.dram_tensor("wq8", [D, HID], f8, kind="ExternalInput")
    wk8 = nc.dram_tensor("wk8", [D, HID], f8, kind="ExternalInput")
    wv8 = nc.dram_tensor("wv8", [D, HID], f8, kind="ExternalInput")
    wvoh = nc.dram_tensor("wvoh", [D, D], f8, kind="ExternalInput")
    wvol = nc.dram_tensor("wvol", [D, D], f8, kind="ExternalInput")
    wo8 = nc.dram_tensor("wo8", [HID, D], f8, kind="ExternalInput")
    out_t = nc.dram_tensor("out_t", [D, TPC], f16, kind="ExternalOutput")

    # two-subtile DR stationary: sub0 = tri01 (k<=q), sub1 = 64/(q+1)*tri01,
    # both 2-head block-diagonal over the 128 partitions.
    tri01 = np.triu(np.ones((C, C), np.float32))          # [k, q]
    qp1 = np.arange(1, C + 1, dtype=np.float32)
    triN = tri01 * (64.0 / qp1)[None, :]

    def blk(m):
        b = np.zeros((128, 128), np.float32)
        b[:C, :C] = m
        b[C:, C:] = m
        return b

    import ml_dtypes
    tri_np = np.stack([blk(tri01), blk(triN)]).astype(ml_dtypes.float8_e4m3)
    tri_d = nc.inline_tensor(tri_np, name="tri")
    ident_np = np.eye(128, dtype=np.float16)
    ident_d = nc.inline_tensor(ident_np, name="ident")

    g = 1.0 / (C0 * qp1.astype(np.float64))
    ca = np.sqrt(1.0 / 2048.0)
    sc1_np = np.tile((np.sqrt(g) * ca).astype(np.float32), 2).reshape(128, 1)
    sc2_np = np.tile((g / 32.0).astype(np.float32), 2).reshape(128, 1)
    sc1_d = nc.inline_tensor(sc1_np, name="sc1")
    sc2_d = nc.inline_tensor(sc2_np, name="sc2")

    with tile.TileContext(nc) as tc:
        with (
            tc.tile_pool(name="res", bufs=1) as res,
            tc.tile_pool(name="qkv8p", bufs=4) as qkv8p,
            tc.tile_pool(name="kv8p", bufs=3) as kv8p,
            tc.tile_pool(name="trp", bufs=4) as trp,
            tc.tile_pool(name="mdp", bufs=4) as mdp,
            tc.tile_pool(name="osb", bufs=6) as osbp,
            tc.tile_pool(name="psQK", bufs=1, space="PSUM") as psQK,     # 2 banks
            tc.tile_pool(name="psV", bufs=1, space="PSUM") as psV,       # 1 bank
            tc.tile_pool(name="psTri", bufs=1, space="PSUM") as psTri,   # 3 banks
            tc.tile_pool(name="psY", bufs=2, space="PSUM") as psY,       # 2 banks
        ):
            # ---- resident loads (SP queue), ordered by first use.
            def load3(dst, dram_ap, eng=None):
                (eng or nc.sync).dma_start(
                    dst[:, :, :],
                    dram_ap.rearrange("(dc p) f -> p dc f", p=128),
                )

            def load3h(dst, dram_ap, half, eng=None):
                F = dst.shape[2]
                fsl = slice(half * F // 2, (half + 1) * F // 2)
                (eng or nc.sync).dma_start(
                    dst[:, :, fsl],
                    dram_ap.rearrange("(dc p) f -> p dc f", p=128)[:, :, fsl],
                )

            def load3q(dst, dram_ap, half, dclo):
                F = dst.shape[2]
                fsl = slice(half * F // 2, (half + 1) * F // 2)
                dsl = slice(0, ND // 2) if dclo else slice(ND // 2, ND)
                nc.sync.dma_start(
                    dst[:, dsl, fsl],
                    dram_ap.rearrange("(dc p) f -> p dc f", p=128)[:, dsl, fsl],
                )

            x8_sb = res.tile([128, ND, TPC], f8, tag="x8", name="x8")
            xr8_sb = res.tile([128, ND, TPC], f8, tag="xr8", name="xr8")
            wq_sb = res.tile([128, ND, HID], f8, tag="wq", name="wq")
            wk_sb = res.tile([128, ND, HID], f8, tag="wk", name="wk")
            wv_sb = res.tile([128, ND, HID], f8, tag="wv", name="wv")
            wvoh_sb = res.tile([128, ND, D], f8, tag="wvoh", name="wvoh")
            wvol_sb = res.tile([128, ND, D], f8, tag="wvol", name="wvol")
            wo8_sb = res.tile([128, ND, D], f8, tag="wo8", name="wo8")

            # iteration-0 critical set first (quarter DMAs)
            load3q(x8_sb, xt8[:, :], 0, True)
            load3q(wq_sb, wq8[:, :], 0, True)
            load3q(x8_sb, xt8[:, :], 0, False)
            load3q(wq_sb, wq8[:, :], 0, False)
            load3h(wk_sb, wk8[:, :], 0)
            load3h(wv_sb, wv8[:, :], 0)
            # consts for the trio matmuls (needed slot 1)
            tri_sb = res.tile([128, 2, 128], f8, tag="tri", name="tri_sb")
            nc.sync.dma_start(tri_sb[:, :, :],
                              tri_d[:, :, :].rearrange("s k q -> k s q"))
            sc1_sb = res.tile([128, 1], f32, tag="sc1", name="sc1_sb")
            nc.sync.dma_start(sc1_sb[:], sc1_d[:, :])
            sc2_sb = res.tile([128, 1], f32, tag="sc2", name="sc2_sb")
            nc.sync.dma_start(sc2_sb[:], sc2_d[:, :])
            ident_sb = res.tile([128, 128], f16, tag="ident", name="ident_sb")
            nc.sync.dma_start(ident_sb[:, :], ident_d[:, :])
            # transfers serialize on the DMA pipe (~1.46us per 0.5MB), so
            # this order IS the arrival schedule.  Slots 0..7 use only the
            # h0 weight halves (cch-interleaved its order), so x8-t1 and the
            # out-path tensors ship next and the h1 halves follow.
            load3h(x8_sb, xt8[:, :], 1)
            load3(wvoh_sb, wvoh[:, :])
            load3(wvol_sb, wvol[:, :])
            load3h(xr8_sb, xtr8[:, :], 1)

            def loadcc(dst, dram_ap, cclo, cchi):
                fsl = slice(cclo * 128, cchi * 128)
                nc.sync.dma_start(
                    dst[:, :, fsl],
                    dram_ap.rearrange("(dc p) f -> p dc f", p=128)[:, :, fsl],
                )
            for c4 in (4, 6):
                loadcc(wq_sb, wq8[:, :], c4, c4 + 2)
                loadcc(wk_sb, wk8[:, :], c4, c4 + 2)
                loadcc(wv_sb, wv8[:, :], c4, c4 + 2)
            load3(wo8_sb, wo8[:, :])
            load3h(xr8_sb, xtr8[:, :], 0)

            # delta tiles per t-chunk, a016 for the t=1 A0 precompute
            delta_sb = [
                res.tile([128, NCH, TCH], f8, tag=f"d{t}", name=f"delta{t}")
                for t in range(NT)
            ]
            a016 = res.tile([128, ND, TCH], f16, tag="a016", name="a016")

            # qkv8 / kv8 ping-pong tiles with static zero pads: grab both
            # rotations once, memset the pads, then rotate normally.
            boot_qkv = [qkv8p.tile([128, 2048], f8, tag="qkv8", name=f"qkvb{i}")
                        for i in range(4)]
            boot_kv = [kv8p.tile([128, 1024], f8, tag="kv8", name=f"kvb{i}")
                       for i in range(3)]
            for bt in boot_qkv:
                nc.gpsimd.memset(bt[:, 1536:2048], 0)
            for bt in boot_kv:
                nc.gpsimd.memset(bt[:, 512:1024], 0)

            def stage_proj(t, cch):
                """Q/K/V fp8 DR projections.  Q|K and V use separate PSUM
                pools so the V drain (DVE) is off the QK recycle path."""
                tsl = slice(t * TCH, (t + 1) * TCH)
                csl = slice(cch * 128, (cch + 1) * 128)
                qp = psQK.tile([128, 2 * TCH], f32, tag="qk", name="qk_ps")
                vp = psV.tile([128, TCH], f32, tag="v", name="v_ps")
                for half, wsb in ((0, wq_sb), (1, wk_sb)):
                    for j, dc in enumerate(range(0, ND, 2)):
                        nc.tensor.matmul(
                            qp[:, half * TCH:(half + 1) * TCH],
                            lhsT=wsb[:, dc:dc + 2, csl],
                            rhs=x8_sb[:, dc:dc + 2, tsl],
                            start=(j == 0),
                            stop=(j == ND // 2 - 1),
                            perf_mode=DR,
                        )
                for j, dc in enumerate(range(0, ND, 2)):
                    nc.tensor.matmul(
                        vp[:],
                        lhsT=wv_sb[:, dc:dc + 2, csl],
                        rhs=x8_sb[:, dc:dc + 2, tsl],
                        start=(j == 0),
                        stop=(j == ND // 2 - 1),
                        perf_mode=DR,
                    )
                return qp, vp

            def stage_qkv_copy(qp, vp):
                """Split fp8 drain: [Q8|K8] on ACT, [V8] on DVE in parallel."""
                qt = qkv8p.tile([128, 2048], f8, tag="qkv8", name="qkv8")
                nc.scalar.mul(qt[:, 0:1024], qp[:], 0.5)
                nc.vector.tensor_scalar_mul(qt[:, 1024:1536], vp[:], 0.5)
                return qt

            def stage_kv(qt):
                kt = kv8p.tile([128, 1024], f8, tag="kv8", name="kv8")
                nc.gpsimd.tensor_mul(kt[:, 0:512], qt[:, 512:1024], qt[:, 1024:1536])
                return kt

            def stage_trio(qt, kt):
                """[S1|P0|P1] prefix sums: 3 fp8 DR matmuls, each using one
                subtile of the [tri01; triN64] stationary via zero-pads."""
                tp = psTri.tile([128, 3 * TCH], f32, tag="tri", name="tri_ps")
                # S1 = tri01 @ K8: sub0 = K8 (cols 512:1024), sub1 = zeros (1536:)
                nc.tensor.matmul(
                    tp[:, 0:TCH], lhsT=tri_sb[:, :, :],
                    rhs=_sub2(qt[:, 512:1024], 1024),
                    start=True, stop=True, perf_mode=DR,
                )
                # P0 = triN64 @ V8: sub0 = zeros (1536:), sub1 = V8 (1024:1536)
                nc.tensor.matmul(
                    tp[:, TCH:2 * TCH], lhsT=tri_sb[:, :, :],
                    rhs=_sub2(qt[:, 1536:2048], -512),
                    start=True, stop=True, perf_mode=DR,
                )
                # P1 = tri01 @ kv8: sub0 = kv8, sub1 = zeros (512:1024)
                nc.tensor.matmul(
                    tp[:, 2 * TCH:3 * TCH], lhsT=tri_sb[:, :, :],
                    rhs=_sub2(kt[:, 0:512], 512),
                    start=True, stop=True, perf_mode=DR,
                )
                return tp

            def stage_trio_copy(tp):
                """Scaled fp16 drains: [S1d|P0d] = sc1*psum (DVE),
                P1d = sc2*psum (ACT)."""
                t1 = trp.tile([128, 2 * TCH], f16, tag="t1", name="t1")
                nc.vector.tensor_scalar_mul(t1[:], tp[:, 0:2 * TCH], sc1_sb[:, 0:1])
                t2 = trp.tile([128, TCH], f16, tag="t2", name="t2")
                nc.scalar.mul(t2[:], tp[:, 2 * TCH:3 * TCH], sc2_sb[:, 0:1])
                return t1, t2

            def stage_md(t1, t2):
                mm = mdp.tile([128, TCH], f16, tag="mm", name="mm")
                nc.vector.tensor_mul(mm[:], t1[:, 0:TCH], t1[:, TCH:2 * TCH])
                dd = mdp.tile([128, TCH], f16, tag="dd", name="dd")
                nc.vector.tensor_sub(dd[:], t2[:], mm[:])
                return dd

            def stage_delta(t, cch, qt, dd):
                """delta8 = Q8 * d  (Pool, fp8 out) into the per-t tile."""
                nc.gpsimd.tensor_mul(delta_sb[t][:, cch, :], qt[:, 0:512], dd[:])

            def a0_mms(t, dc, yp, stop):
                """x8@wvo_hi + x8@wvo_lo + xr8@wvo_hi into yp."""
                tsl = slice(t * TCH, (t + 1) * TCH)
                dcsl = slice(dc * 128, (dc + 1) * 128)
                streams = ((wvoh_sb, x8_sb), (wvol_sb, x8_sb), (wvoh_sb, xr8_sb))
                for si, (wsb, xs) in enumerate(streams):
                    for j, dcc in enumerate(range(0, ND, 2)):
                        nc.tensor.matmul(
                            yp[:],
                            lhsT=wsb[:, dcc:dcc + 2, dcsl],
                            rhs=xs[:, dcc:dcc + 2, tsl],
                            start=(si == 0 and j == 0),
                            stop=(stop and si == 2 and j == ND // 2 - 1),
                            perf_mode=DR,
                        )

            def delta_mms(t, dc, yp, start, stop=True):
                dcsl = slice(dc * 128, (dc + 1) * 128)
                for j, hc in enumerate(range(0, NCH, 2)):
                    nc.tensor.matmul(
                        yp[:],
                        lhsT=wo8_sb[:, hc:hc + 2, dcsl],
                        rhs=delta_sb[t][:, hc:hc + 2, :],
                        start=(start and j == 0),
                        stop=(stop and j == NCH // 2 - 1),
                        perf_mode=DR,
                    )

            def piece_a0_sbuf(dc):
                """Precompute A0 for t=1 into a016[dc] (tail uses DVE add)."""
                yp = psY.tile([128, TCH], f32, tag="y", name="y_a0")
                a0_mms(1, dc, yp, stop=True)
                nc.scalar.copy(a016[:, dc, :], yp[:])

            def piece_y_fused(dc, alt):
                """Full fused out group for t=0: A0 + delta -> copy -> DMA."""
                yp = psY.tile([128, TCH], f32, tag="y", name="y_f")
                a0_mms(0, dc, yp, stop=False)
                delta_mms(0, dc, yp, start=False)
                ot = osbp.tile([128, TCH], f16, tag="ot", name="ot")
                nc.scalar.copy(ot[:], yp[:])
                deng = nc.sync if alt else nc.scalar
                deng.dma_start(out_t[dc * 128:(dc + 1) * 128, 0:TCH], ot[:])

            def piece_y_tail(dc, slot_ap):
                """Tail group for t=1: delta matmuls + a016 pre-added on the
                PE via an fp16 identity matmul, so the drain is a plain copy
                (alternating ACT/DVE) instead of a serialized DVE add.  The
                projection/trio psum pools are retired by now — slot_ap may
                borrow their banks so all 8 groups pipeline."""
                dcsl = slice(dc * 128, (dc + 1) * 128)
                for j, hc in enumerate(range(0, NCH, 2)):
                    nc.tensor.matmul(
                        slot_ap,
                        lhsT=wo8_sb[:, hc:hc + 2, dcsl],
                        rhs=delta_sb[1][:, hc:hc + 2, :],
                        start=(j == 0),
                        stop=False,
                        perf_mode=DR,
                    )
                nc.tensor.matmul(
                    slot_ap, lhsT=ident_sb[:, :], rhs=a016[:, dc, :],
                    start=False, stop=True,
                )
                ot = osbp.tile([128, TCH], f16, tag="ot", name="ot")
                if dc % 2 == 0:
                    nc.scalar.copy(ot[:], slot_ap)
                else:
                    nc.vector.tensor_copy(ot[:], slot_ap)
                deng = nc.sync if dc % 2 == 0 else nc.scalar
                deng.dma_start(out_t[dc * 128:(dc + 1) * 128, TCH:2 * TCH], ot[:])

            # ---- main pipeline ----
            its = ([(0, c) for c in range(4)] + [(1, c) for c in range(4)] +
                   [(0, c) for c in range(4, 8)] + [(1, c) for c in range(4, 8)])
            n = len(its)
            ctx = {}
            # out-projection pieces: a0 precompute gated on its DMA loads
            # (~slot 5), fused groups gated on delta[0] completeness (slot 9)
            pending = [("a0", dc) for dc in range(ND)] + \
                      [("fused", dc) for dc in range(ND)]
            nalt = [0]

            def pop_piece(idx):
                if not pending:
                    return False
                kind, dc = pending[0]
                if kind == "a0" and idx < 6:
                    return False
                if kind == "fused" and idx < 13:
                    return False
                pending.pop(0)
                if kind == "a0":
                    piece_a0_sbuf(dc)
                else:
                    nalt[0] += 1
                    piece_y_fused(dc, alt=(nalt[0] % 2 == 0))
                return True

            for idx in range(n + 1):
                if idx < n:
                    qp, vp = stage_proj(*its[idx])
                    qt = stage_qkv_copy(qp, vp)
                    kt = stage_kv(qt)
                    ctx[idx] = (qt, kt)

                if idx >= 1:
                    pqt, pkt = ctx.pop(idx - 1)
                    tp = stage_trio(pqt, pkt)
                    t1, t2 = stage_trio_copy(tp)
                    dd = stage_md(t1, t2)
                    t_, cch_ = its[idx - 1]
                    stage_delta(t_, cch_, pqt, dd)
                # pace: a0 spread over slots 6..12, fused 2x over 13..16
                npop = {6: 2, 13: 2, 14: 2, 15: 2, 16: 2}.get(idx, 1)
                for _ in range(npop):
                    if not pop_piece(idx):
                        break
            while pending:
                kind, dc = pending.pop(0)
                assert kind == "fused"
                nalt[0] += 1
                piece_y_fused(dc, alt=(nalt[0] % 2 == 0))
            # tail psum slots: 2 from psY, 2 from psQK, 3 from psTri, 1 psY
            yt0 = psY.tile([128, TCH], f32, tag="y", name="yt0")
            yt1 = psY.tile([128, TCH], f32, tag="y", name="yt1")
            qt_ = psQK.tile([128, 2 * TCH], f32, tag="qk", name="yt_qk")
            tt_ = psTri.tile([128, 3 * TCH], f32, tag="tri", name="yt_tri")
            vt_ = psV.tile([128, TCH], f32, tag="v", name="yt_v")
            slots = [yt0[:], yt1[:], qt_[:, 0:TCH], qt_[:, TCH:2 * TCH],
                     tt_[:, 0:TCH], tt_[:, TCH:2 * TCH], tt_[:, 2 * TCH:3 * TCH],
                     vt_[:]]
            for dc in range(ND):
                piece_y_tail(dc, slots[dc])

    nc.finalize()
    return nc


def _get_nc():
    if "nc" not in _CACHE:
        _CACHE["nc"] = _build_bass()
    return _CACHE["nc"]


def _e4m3(x):
    import ml_dtypes
    return np.clip(x, -240.0, 240.0).astype(ml_dtypes.float8_e4m3)


def _make_in_maps(x, Wq, bq, Wk, bk, Wv, bv, Wo, bo):
    for b in (bq, bk, bv, bo):
        assert not np.any(np.asarray(b)), "nonzero biases not supported"
    x_flat = np.ascontiguousarray(x, dtype=np.float32).reshape(TOK, D)
    Wq = np.ascontiguousarray(Wq, dtype=np.float32)
    Wk = np.ascontiguousarray(Wk, dtype=np.float32)
    Wv = np.ascontiguousarray(Wv, dtype=np.float32)
    Wo = np.ascontiguousarray(Wo, dtype=np.float32)

    # Wvo = Wv @ blockdiag(M)^T @ Wo, M[q,k] = [k<=q]/(q+1)
    M = np.tril(np.ones((C, C), np.float64)) / np.arange(1, C + 1)[:, None]
    BD = np.zeros((HID, HID), np.float64)
    for h in range(H):
        BD[h * C:(h + 1) * C, h * C:(h + 1) * C] = M.T
    Wvo = (Wv.astype(np.float64) @ (BD @ Wo.astype(np.float64))).astype(np.float32)

    wq8 = _e4m3(8.0 * Wq)
    wk8 = _e4m3(64.0 * C1 * Wk)
    wv8 = _e4m3(2.0 * Wv)
    wvoh = _e4m3(WVOS * Wvo)
    wvol = _e4m3(WVOS * Wvo - wvoh.astype(np.float32))
    wo8 = _e4m3((WVOS / 32.0) * Wo)

    in_maps = []
    for i in range(NCORES):
        shard = x_flat[i * TPC:(i + 1) * TPC]
        xt = np.ascontiguousarray(shard.T)
        xt8 = _e4m3(xt)
        xtr8 = _e4m3(xt - xt8.astype(np.float32))
        in_maps.append({
            "xt8": xt8, "xtr8": xtr8,
            "wq8": wq8, "wk8": wk8, "wv8": wv8,
            "wvoh": wvoh, "wvol": wvol, "wo8": wo8,
        })
    return in_maps


def _run(in_maps, trace=False, **kw):
    from concourse import bass_utils
    nc = _get_nc()
    res = bass_utils.run_bass_kernel_spmd(
        nc, in_maps, core_ids=list(range(NCORES)), trace=trace, **kw
    )
    return res


def kernel(x, Wq, bq, Wk, bk, Wv, bv, Wo, bo):
    in_maps = _make_in_maps(x, Wq, bq, Wk, bk, Wv, bv, Wo, bo)
    out = np.empty((TOK, D), np.float32)
    for attempt in range(2):
        res = _run(in_maps, trace=False)
        for i in range(NCORES):
            out[i * TPC:(i + 1) * TPC] = (
                res.results[i]["out_t"].T.astype(np.float32) / WVOS
            )
        if np.isfinite(out).all():
            break
    return out.reshape(B, S, D)


# revision 36
# speedup vs baseline: 1.2228x; 1.0692x over previous
"""Trainium2 Bass kernel for nn_AttentionModule_30021821399395 — v4.

Math (per token, head h; C=64 channels, degree-1 exp poly c0 + c1*s):
  attn[c_q] = N/Z with N = c0*P0 + c1*Q o P1, Z = c0(q+1) + c1*Q o S1,
  (P0, P1, S1) = prefix sums over k<=c_q of (V, K*V, K).

v4 delta-decomposition: split attn = A0 + delta where
  A0[c_q] = mean_{k<=c_q} V[k]  (the dominant ~93% part) is LINEAR in V, so
  y_A0 = x @ Wvo with Wvo = Wv @ blockdiag(M)^T @ Wo precomputed on host
  (M[q,k] = [k<=q]/(q+1)).  The on-chip work is only the small correction
  delta = c1 * Q o (P1 - P0 o S1/(q+1)) / Z  (~7% magnitude), for which
  1/Z ~= 1/(c0(q+1)) (a <=5% perturbation of a 7% term; validated 4.6e-3).

Everything then runs in fp8 E4M3 DoubleRow (0.5 cyc/row vs fp16's 1.0):
  - Q/K/V projections: 1 fp8 stream each, drained by ONE wide fp8 copy
    (scales folded into host weights so Q8=32Qs, K8=32c1K, V8=V share
    one 0.5 copy scale).
  - prefix sums: one fp8 DR matmul each with a [tri01; triN64] two-subtile
    stationary; the unused subtile reads a static zero region via a
    stride-surgery AP.  Per-channel scale vectors sqrt(g)*ca and g/32 on
    the PSUM drains make m = S1d*P0d and d = P1d - m come out at
    delta8 = 32*delta directly (g = 1/(c0(q+1))).
  - out: y = x@wvo_hi + x@wvo_lo + xr8@wvo_hi + delta8@wo8 accumulated in
    one PSUM bank (A0 needs 3 streams: 2-stream fails at 2.7e-2).  Output
    is WVOS-scaled fp16; host divides by WVOS.

Schedule: 16 slots of (t,cch) projection pipeline; slot s also carries one
out-projection piece: slots 1..8 precompute A0 for token-chunk t=1 into
SBUF (a016), slots 9..16 run fused A0+delta groups for t=0, and the tail
runs t=1's delta matmuls + DVE add with a016.

Sharding: data-parallel over the 8192 tokens -> 1024 tokens/core x 8.
numpy oracle rel err 4.63e-3 (budget 2e-2).
"""

import sys

if "/opt/trn_rl_repo" not in sys.path:
    sys.path.insert(0, "/opt/trn_rl_repo")

import numpy as np

B, S, D = 4, 2048, 1024
H, C = 16, 64
HID = H * C
NCORES = 8
TOK = B * S
TPC = TOK // NCORES         # 1024 tokens per core
TCH = 512                   # token chunk
NT = TPC // TCH             # 2
NCH = HID // 128            # 8 hid chunks
ND = D // 128               # 8 contraction chunks

COEF = np.array([1.0013245, 1.0040334], dtype=np.float64)
C0, C1 = float(COEF[0]), float(COEF[1])
WVOS = 4096.0

_CACHE = {}


def _sub2(ap, stride):
    """[128, N] AP -> [128, 2, N] with a custom middle-dim stride, selecting
    the real data subtile and a static-zero subtile for DoubleRow."""
    a = list(ap.ap)
    assert len(a) == 2, a
    new = [list(a[0]), [stride, 2], list(a[1])]
    return type(ap)(ap.tensor, ap.offset, new)


def _build_bass():
    import concourse.mybir as mybir
    import concourse.tile as tile
    from concourse import bacc

    f32 = mybir.dt.float32
    f16 = mybir.dt.float16
    f8 = mybir.dt.float8e4
    DR = mybir.MatmulPerfMode.DoubleRow

    nc = bacc.Bacc("TRN2")

    xt8 = nc.dram_tensor("xt8", [D, TPC], f8, kind="ExternalInput")
    xtr8 = nc.dram_tensor("xtr8", [D, TPC], f8, kind="ExternalInput")
    wq8 = nc.dram_tensor("wq8", [D, HID], f8, kind="ExternalInput")
    wk8 = nc.dram_tensor("wk8", [D, HID], f8, kind="ExternalInput")
    wv8 = nc.dram_tensor("wv8", [D, HID], f8, kind="ExternalInput")
    wvoh = nc.dram_tensor("wvoh", [D, D], f8, kind="ExternalInput")
    wvol = nc.dram_tensor("wvol", [D, D], f8, kind="ExternalInput")
    wo8 = nc.dram_tensor("wo8", [HID, D], f8, kind="ExternalInput")
    out_t = nc.dram_tensor("out_t", [D, TPC], f16, kind="ExternalOutput")

    # two-subtile DR stationary: sub0 = tri01 (k<=q), sub1 = 64/(q+1)*tri01,
    # both 2-head block-diagonal over the 128 partitions.
    tri01 = np.triu(np.ones((C, C), np.float32))          # [k, q]
    qp1 = np.arange(1, C + 1, dtype=np.float32)
    triN = tri01 * (64.0 / qp1)[None, :]

    def blk(m):
        b = np.zeros((128, 128), np.float32)
        b[:C, :C] = m
        b[C:, C:] = m
        return b

    import ml_dtypes
    tri_np = np.stack([blk(tri01), blk(triN)]).astype(ml_dtypes.float8_e4m3)
    tri_d = nc.inline_tensor(tri_np, name="tri")
    ident_np = np.eye(128, dtype=np.float16)
    ident_d = nc.inline_tensor(ident_np, name="ident")

    g = 1.0 / (C0 * qp1.astype(np.float64))
    ca = np.sqrt(1.0 / 2048.0)
    sc1_np = np.tile((np.sqrt(g) * ca).astype(np.float32), 2).reshape(128, 1)
    sc2_np = np.tile((g / 32.0).astype(np.float32), 2).reshape(128, 1)
    sc1_d = nc.inline_tensor(sc1_np, name="sc1")
    sc2_d = nc.inline_tensor(sc2_np, name="sc2")

    with tile.TileContext(nc) as tc:
        with (
            tc.tile_pool(name="res", bufs=1) as res,
            tc.tile_pool(name="qkv8p", bufs=4) as qkv8p,
            tc.tile_pool(name="kv8p", bufs=3) as kv8p,
            tc.tile_pool(name="trp", bufs=4) as trp,
            tc.tile_pool(name="mdp", bufs=4) as mdp,
            tc.tile_pool(name="osb", bufs=6) as osbp,
            tc.tile_pool(name="psQK", bufs=1, space="PSUM") as psQK,     # 2 banks
            tc.tile_pool(name="psV", bufs=1, space="PSUM") as psV,       # 1 bank
            tc.tile_pool(name="psTri", bufs=1, space="PSUM") as psTri,   # 3 banks
            tc.tile_pool(name="psY", bufs=2, space="PSUM") as psY,       # 2 banks
        ):
            # ---- resident loads (SP queue), ordered by first use.
            def load3(dst, dram_ap, eng=None):
                (eng or nc.sync).dma_start(
                    dst[:, :, :],
                    dram_ap.rearrange("(dc p) f -> p dc f", p=128),
                )

            def load3h(dst, dram_ap, half, eng=None):
                F = dst.shape[2]
                fsl = slice(half * F // 2, (half + 1) * F // 2)
                (eng or nc.sync).dma_start(
                    dst[:, :, fsl],
                    dram_ap.rearrange("(dc p) f -> p dc f", p=128)[:, :, fsl],
                )

            def load3q(dst, dram_ap, half, dclo):
                F = dst.shape[2]
                fsl = slice(half * F // 2, (half + 1) * F // 2)
                dsl = slice(0, ND // 2) if dclo else slice(ND // 2, ND)
                nc.sync.dma_start(
                    dst[:, dsl, fsl],
                    dram_ap.rearrange("(dc p) f -> p dc f", p=128)[:, dsl, fsl],
                )

            x8_sb = res.tile([128, ND, TPC], f8, tag="x8", name="x8")
            xr8_sb = res.tile([128, ND, TPC], f8, tag="xr8", name="xr8")
            wq_sb = res.tile([128, ND, HID], f8, tag="wq", name="wq")
            wk_sb = res.tile([128, ND, HID], f8, tag="wk", name="wk")
            wv_sb = res.tile([128, ND, HID], f8, tag="wv", name="wv")
            wvoh_sb = res.tile([128, ND, D], f8, tag="wvoh", name="wvoh")
            wvol_sb = res.tile([128, ND, D], f8, tag="wvol", name="wvol")
            wo8_sb = res.tile([128, ND, D], f8, tag="wo8", name="wo8")

            # iteration-0 critical set first (quarter DMAs)
            load3q(x8_sb, xt8[:, :], 0, True)
            load3q(wq_sb, wq8[:, :], 0, True)
            load3q(x8_sb, xt8[:, :], 0, False)
            load3q(wq_sb, wq8[:, :], 0, False)
            load3h(wk_sb, wk8[:, :], 0)
            load3h(wv_sb, wv8[:, :], 0)
            # consts for the trio matmuls (needed slot 1)
            tri_sb = res.tile([128, 2, 128], f8, tag="tri", name="tri_sb")
            nc.sync.dma_start(tri_sb[:, :, :],
                              tri_d[:, :, :].rearrange("s k q -> k s q"))
            sc1_sb = res.tile([128, 1], f32, tag="sc1", name="sc1_sb")
            nc.sync.dma_start(sc1_sb[:], sc1_d[:, :])
            sc2_sb = res.tile([128, 1], f32, tag="sc2", name="sc2_sb")
            nc.sync.dma_start(sc2_sb[:], sc2_d[:, :])
            ident_sb = res.tile([128, 128], f16, tag="ident", name="ident_sb")
            nc.sync.dma_start(ident_sb[:, :], ident_d[:, :])
            # transfers serialize on the DMA pipe (~1.46us per 0.5MB), so
            # this order IS the arrival schedule.  Slots 0..7 use only the
            # h0 weight halves (cch-interleaved its order), so x8-t1 and the
            # out-path tensors ship next and the h1 halves follow.
            load3h(x8_sb, xt8[:, :], 1)
            load3(wvoh_sb, wvoh[:, :])
            load3(wvol_sb, wvol[:, :])
            load3h(xr8_sb, xtr8[:, :], 1)

            def loadcc(dst, dram_ap, cclo, cchi):
                fsl = slice(cclo * 128, cchi * 128)
                nc.sync.dma_start(
                    dst[:, :, fsl],
                    dram_ap.rearrange("(dc p) f -> p dc f", p=128)[:, :, fsl],
                )
            for c4 in (4, 6):
                loadcc(wq_sb, wq8[:, :], c4, c4 + 2)
                loadcc(wk_sb, wk8[:, :], c4, c4 + 2)
                loadcc(wv_sb, wv8[:, :], c4, c4 + 2)
            load3(wo8_sb, wo8[:, :])
            load3h(xr8_sb, xtr8[:, :], 0)

            # delta tiles per t-chunk, a016 for the t=1 A0 precompute
            delta_sb = [
                res.tile([128, NCH, TCH], f8, tag=f"d{t}", name=f"delta{t}")
                for t in range(NT)
            ]
            a016 = res.tile([128, ND, TCH], f16, tag="a016", name="a016")

            # qkv8 / kv8 ping-pong tiles with static zero pads: grab both
            # rotations once, memset the pads, then rotate normally.
            boot_qkv = [qkv8p.tile([128, 2048], f8, tag="qkv8", name=f"qkvb{i}")
                        for i in range(4)]
            boot_kv = [kv8p.tile([128, 1024], f8, tag="kv8", name=f"kvb{i}")
                       for i in range(3)]
            # dummy matmul on a tiny zeroed pad at ~0.3us: starts the PE
            # p-state ramp clock so every real matmul (first at ~4.4us) runs
            # at full frequency (ramp reaches max 3us after first activity).
            nc.gpsimd.memset(boot_kv[0][:, 512:768], 0)
            warm = psY.tile([128, 128], f32, tag="y", name="warm")
            nc.tensor.matmul(
                warm[:], lhsT=boot_kv[0][:, 512:640],
                rhs=boot_kv[0][:, 640:768], start=True, stop=True,
            )
            for bt in boot_qkv:
                nc.gpsimd.memset(bt[:, 1536:2048], 0)
            for bt in boot_kv:
                nc.gpsimd.memset(bt[:, 512:1024], 0)

            def stage_proj(t, cch):
                """Q/K/V fp8 DR projections.  Q|K and V use separate PSUM
                pools so the V drain (DVE) is off the QK recycle path."""
                tsl = slice(t * TCH, (t + 1) * TCH)
                csl = slice(cch * 128, (cch + 1) * 128)
                qp = psQK.tile([128, 2 * TCH], f32, tag="qk", name="qk_ps")
                vp = psV.tile([128, TCH], f32, tag="v", name="v_ps")
                for half, wsb in ((0, wq_sb), (1, wk_sb)):
                    for j, dc in enumerate(range(0, ND, 2)):
                        nc.tensor.matmul(
                            qp[:, half * TCH:(half + 1) * TCH],
                            lhsT=wsb[:, dc:dc + 2, csl],
                            rhs=x8_sb[:, dc:dc + 2, tsl],
                            start=(j == 0),
                            stop=(j == ND // 2 - 1),
                            perf_mode=DR,
                        )
                for j, dc in enumerate(range(0, ND, 2)):
                    nc.tensor.matmul(
                        vp[:],
                        lhsT=wv_sb[:, dc:dc + 2, csl],
                        rhs=x8_sb[:, dc:dc + 2, tsl],
                        start=(j == 0),
                        stop=(j == ND // 2 - 1),
                        perf_mode=DR,
                    )
                return qp, vp

            def stage_qkv_copy(qp, vp):
                """Split fp8 drain: [Q8|K8] on ACT, [V8] on DVE in parallel."""
                qt = qkv8p.tile([128, 2048], f8, tag="qkv8", name="qkv8")
                nc.scalar.mul(qt[:, 0:1024], qp[:], 0.5)
                nc.vector.tensor_scalar_mul(qt[:, 1024:1536], vp[:], 0.5)
                return qt

            def stage_kv(qt):
                kt = kv8p.tile([128, 1024], f8, tag="kv8", name="kv8")
                nc.gpsimd.tensor_mul(kt[:, 0:512], qt[:, 512:1024], qt[:, 1024:1536])
                return kt

            def stage_trio(qt, kt):
                """[S1|P0|P1] prefix sums: 3 fp8 DR matmuls, each using one
                subtile of the [tri01; triN64] stationary via zero-pads."""
                tp = psTri.tile([128, 3 * TCH], f32, tag="tri", name="tri_ps")
                # S1 = tri01 @ K8: sub0 = K8 (cols 512:1024), sub1 = zeros (1536:)
                nc.tensor.matmul(
                    tp[:, 0:TCH], lhsT=tri_sb[:, :, :],
                    rhs=_sub2(qt[:, 512:1024], 1024),
                    start=True, stop=True, perf_mode=DR,
                )
                # P0 = triN64 @ V8: sub0 = zeros (1536:), sub1 = V8 (1024:1536)
                nc.tensor.matmul(
                    tp[:, TCH:2 * TCH], lhsT=tri_sb[:, :, :],
                    rhs=_sub2(qt[:, 1536:2048], -512),
                    start=True, stop=True, perf_mode=DR,
                )
                # P1 = tri01 @ kv8: sub0 = kv8, sub1 = zeros (512:1024)
                nc.tensor.matmul(
                    tp[:, 2 * TCH:3 * TCH], lhsT=tri_sb[:, :, :],
                    rhs=_sub2(kt[:, 0:512], 512),
                    start=True, stop=True, perf_mode=DR,
                )
                return tp

            def stage_trio_copy(tp, fast=False):
                """Scaled fp16 drains: [S1d|P0d] = sc1*psum (DVE),
                P1d = sc2*psum (ACT).  fast=True (final slots) splits the
                sc1 drain across ACT+DVE to shorten the closing chain."""
                t1 = trp.tile([128, 2 * TCH], f16, tag="t1", name="t1")
                if fast:
                    nc.scalar.mul(t1[:, 0:TCH], tp[:, 0:TCH], sc1_sb[:, 0:1])
                    nc.vector.tensor_scalar_mul(
                        t1[:, TCH:2 * TCH], tp[:, TCH:2 * TCH], sc1_sb[:, 0:1])
                else:
                    nc.vector.tensor_scalar_mul(
                        t1[:], tp[:, 0:2 * TCH], sc1_sb[:, 0:1])
                t2 = trp.tile([128, TCH], f16, tag="t2", name="t2")
                nc.scalar.mul(t2[:], tp[:, 2 * TCH:3 * TCH], sc2_sb[:, 0:1])
                return t1, t2

            def stage_md(t1, t2):
                mm = mdp.tile([128, TCH], f16, tag="mm", name="mm")
                nc.vector.tensor_mul(mm[:], t1[:, 0:TCH], t1[:, TCH:2 * TCH])
                dd = mdp.tile([128, TCH], f16, tag="dd", name="dd")
                nc.vector.tensor_sub(dd[:], t2[:], mm[:])
                return dd

            def stage_delta(t, cch, qt, dd, fast=False):
                """delta8 = Q8 * d into the per-t tile (Pool; DVE on the
                final slots where Pool's queue would gate the tail)."""
                eng = nc.vector if fast else nc.gpsimd
                eng.tensor_mul(delta_sb[t][:, cch, :], qt[:, 0:512], dd[:])

            def a0_mms(t, dc, yp, stop):
                """x8@wvo_hi + x8@wvo_lo + xr8@wvo_hi into yp."""
                tsl = slice(t * TCH, (t + 1) * TCH)
                dcsl = slice(dc * 128, (dc + 1) * 128)
                streams = ((wvoh_sb, x8_sb), (wvol_sb, x8_sb), (wvoh_sb, xr8_sb))
                for si, (wsb, xs) in enumerate(streams):
                    for j, dcc in enumerate(range(0, ND, 2)):
                        nc.tensor.matmul(
                            yp[:],
                            lhsT=wsb[:, dcc:dcc + 2, dcsl],
                            rhs=xs[:, dcc:dcc + 2, tsl],
                            start=(si == 0 and j == 0),
                            stop=(stop and si == 2 and j == ND // 2 - 1),
                            perf_mode=DR,
                        )

            def delta_mms(t, dc, yp, start, stop=True):
                dcsl = slice(dc * 128, (dc + 1) * 128)
                for j, hc in enumerate(range(0, NCH, 2)):
                    nc.tensor.matmul(
                        yp[:],
                        lhsT=wo8_sb[:, hc:hc + 2, dcsl],
                        rhs=delta_sb[t][:, hc:hc + 2, :],
                        start=(start and j == 0),
                        stop=(stop and j == NCH // 2 - 1),
                        perf_mode=DR,
                    )

            def piece_a0_sbuf(dc):
                """Precompute A0 for t=1 into a016[dc] (tail uses DVE add)."""
                yp = psY.tile([128, TCH], f32, tag="y", name="y_a0")
                a0_mms(1, dc, yp, stop=True)
                nc.scalar.copy(a016[:, dc, :], yp[:])

            def piece_y_fused(dc, alt):
                """Full fused out group for t=0: A0 + delta -> copy -> DMA."""
                yp = psY.tile([128, TCH], f32, tag="y", name="y_f")
                a0_mms(0, dc, yp, stop=False)
                delta_mms(0, dc, yp, start=False)
                ot = osbp.tile([128, TCH], f16, tag="ot", name="ot")
                nc.scalar.copy(ot[:], yp[:])
                deng = nc.sync if alt else nc.scalar
                deng.dma_start(out_t[dc * 128:(dc + 1) * 128, 0:TCH], ot[:])

            def piece_y_tail(dc, slot_ap):
                """Tail group for t=1: delta matmuls + a016 pre-added on the
                PE via an fp16 identity matmul, so the drain is a plain copy
                (alternating ACT/DVE) instead of a serialized DVE add.  The
                projection/trio psum pools are retired by now — slot_ap may
                borrow their banks so all 8 groups pipeline."""
                dcsl = slice(dc * 128, (dc + 1) * 128)
                for j, hc in enumerate(range(0, NCH, 2)):
                    nc.tensor.matmul(
                        slot_ap,
                        lhsT=wo8_sb[:, hc:hc + 2, dcsl],
                        rhs=delta_sb[1][:, hc:hc + 2, :],
                        start=(j == 0),
                        stop=False,
                        perf_mode=DR,
                    )
                nc.tensor.matmul(
                    slot_ap, lhsT=ident_sb[:, :], rhs=a016[:, dc, :],
                    start=False, stop=True,
                )
                ot = osbp.tile([128, TCH], f16, tag="ot", name="ot")
                if dc % 2 == 0:
                    nc.scalar.copy(ot[:], slot_ap)
                else:
                    nc.vector.tensor_copy(ot[:], slot_ap)
                deng = nc.sync if dc % 2 == 0 else nc.scalar
                deng.dma_start(out_t[dc * 128:(dc + 1) * 128, TCH:2 * TCH], ot[:])

            # ---- main pipeline ----
            its = ([(0, c) for c in range(4)] + [(1, c) for c in range(4)] +
                   [(0, c) for c in range(4, 8)] + [(1, c) for c in range(4, 8)])
            n = len(its)
            ctx = {}
            # out-projection pieces: a0 precompute gated on its DMA loads
            # (~slot 5), fused groups gated on delta[0] completeness (slot 9)
            pending = [("a0", dc) for dc in range(ND)] + \
                      [("fused", dc) for dc in range(ND)]
            nalt = [0]

            def pop_piece(idx):
                if not pending:
                    return False
                kind, dc = pending[0]
                if kind == "a0" and idx < 6:
                    return False
                if kind == "fused" and idx < 13:
                    return False
                pending.pop(0)
                if kind == "a0":
                    piece_a0_sbuf(dc)
                else:
                    nalt[0] += 1
                    piece_y_fused(dc, alt=(nalt[0] % 2 == 0))
                return True

            for idx in range(n + 1):
                if idx < n:
                    qp, vp = stage_proj(*its[idx])
                    qt = stage_qkv_copy(qp, vp)
                    kt = stage_kv(qt)
                    ctx[idx] = (qt, kt)

                if idx >= 1:
                    fast = (idx >= n - 1)
                    pqt, pkt = ctx.pop(idx - 1)
                    tp = stage_trio(pqt, pkt)
                    t1, t2 = stage_trio_copy(tp, fast=fast)
                    dd = stage_md(t1, t2)
                    t_, cch_ = its[idx - 1]
                    stage_delta(t_, cch_, pqt, dd, fast=fast)
                # pace: a0 spread over slots 6..12, fused 2x over 13..16
                npop = {6: 2, 13: 2, 14: 2, 15: 2, 16: 2}.get(idx, 1)
                for _ in range(npop):
                    if not pop_piece(idx):
                        break
            while pending:
                kind, dc = pending.pop(0)
                assert kind == "fused"
                nalt[0] += 1
                piece_y_fused(dc, alt=(nalt[0] % 2 == 0))
            # tail psum slots: 2 from psY, 2 from psQK, 3 from psTri, 1 psY
            yt0 = psY.tile([128, TCH], f32, tag="y", name="yt0")
            yt1 = psY.tile([128, TCH], f32, tag="y", name="yt1")
            qt_ = psQK.tile([128, 2 * TCH], f32, tag="qk", name="yt_qk")
            tt_ = psTri.tile([128, 3 * TCH], f32, tag="tri", name="yt_tri")
            vt_ = psV.tile([128, TCH], f32, tag="v", name="yt_v")
            slots = [yt0[:], yt1[:], qt_[:, 0:TCH], qt_[:, TCH:2 * TCH],
                     tt_[:, 0:TCH], tt_[:, TCH:2 * TCH], tt_[:, 2 * TCH:3 * TCH],
                     vt_[:]]
            for dc in range(ND):
                piece_y_tail(dc, slots[dc])

    nc.finalize()
    return nc


def _get_nc():
    if "nc" not in _CACHE:
        _CACHE["nc"] = _build_bass()
    return _CACHE["nc"]


def _e4m3(x):
    import ml_dtypes
    return np.clip(x, -240.0, 240.0).astype(ml_dtypes.float8_e4m3)


def _make_in_maps(x, Wq, bq, Wk, bk, Wv, bv, Wo, bo):
    for b in (bq, bk, bv, bo):
        assert not np.any(np.asarray(b)), "nonzero biases not supported"
    x_flat = np.ascontiguousarray(x, dtype=np.float32).reshape(TOK, D)
    Wq = np.ascontiguousarray(Wq, dtype=np.float32)
    Wk = np.ascontiguousarray(Wk, dtype=np.float32)
    Wv = np.ascontiguousarray(Wv, dtype=np.float32)
    Wo = np.ascontiguousarray(Wo, dtype=np.float32)

    # Wvo = Wv @ blockdiag(M)^T @ Wo, M[q,k] = [k<=q]/(q+1)
    M = np.tril(np.ones((C, C), np.float64)) / np.arange(1, C + 1)[:, None]
    BD = np.zeros((HID, HID), np.float64)
    for h in range(H):
        BD[h * C:(h + 1) * C, h * C:(h + 1) * C] = M.T
    Wvo = (Wv.astype(np.float64) @ (BD @ Wo.astype(np.float64))).astype(np.float32)

    wq8 = _e4m3(8.0 * Wq)
    wk8 = _e4m3(64.0 * C1 * Wk)
    wv8 = _e4m3(2.0 * Wv)
    wvoh = _e4m3(WVOS * Wvo)
    wvol = _e4m3(WVOS * Wvo - wvoh.astype(np.float32))
    wo8 = _e4m3((WVOS / 32.0) * Wo)

    in_maps = []
    for i in range(NCORES):
        shard = x_flat[i * TPC:(i + 1) * TPC]
        xt = np.ascontiguousarray(shard.T)
        xt8 = _e4m3(xt)
        xtr8 = _e4m3(xt - xt8.astype(np.float32))
        in_maps.append({
            "xt8": xt8, "xtr8": xtr8,
            "wq8": wq8, "wk8": wk8, "wv8": wv8,
            "wvoh": wvoh, "wvol": wvol, "wo8": wo8,
        })
    return in_maps


def _run(in_maps, trace=False, **kw):
    from concourse import bass_utils
    nc = _get_nc()
    res = bass_utils.run_bass_kernel_spmd(
        nc, in_maps, core_ids=list(range(NCORES)), trace=trace, **kw
    )
    return res


def kernel(x, Wq, bq, Wk, bk, Wv, bv, Wo, bo):
    in_maps = _make_in_maps(x, Wq, bq, Wk, bk, Wv, bv, Wo, bo)
    out = np.empty((TOK, D), np.float32)
    for attempt in range(2):
        res = _run(in_maps, trace=False)
        for i in range(NCORES):
            out[i * TPC:(i + 1) * TPC] = (
                res.results[i]["out_t"].T.astype(np.float32) / WVOS
            )
        if np.isfinite(out).all():
            break
    return out.reshape(B, S, D)
